# revision 17
# baseline (speedup 1.0000x reference)
"""Trainium2 Bass kernel for batched attention-energy softmax.

Computes, for B=64, S=2048, H=1024:
    energy = char_state_decoder @ W.T + b            # [B, H]
    attn   = softmax(einsum("bsh,bh->bs", encoder_outputs, energy), axis=1)
    return attn[:, None, :]                          # [B, 1, S]

Sharding: data-parallel over batch across 8 NeuronCores (8 batches/core).
W/b are replicated; W is pre-transposed+tiled on host (layout only).

Per-core device program:
  1. PE: energy[8,1024] = csd_local @ W.T + b  (K-accumulated matmuls in PSUM)
  2. PE: broadcast energy row b to 128 partitions via one-hot selector matmul
  3. DVE: for each [128s, 1024h] encoder tile, ONE fused tensor_tensor_reduce
     (elementwise mult + free-dim sum) -> attn energies column [128, 1]
  4. Softmax per batch over [128, 16] (s = col*128 + partition):
     DVE free-dim max -> GPSIMD partition all-reduce -> ScalarE Exp with
     bias=-max and accum_out partial sums -> GPSIMD all-reduce -> DVE
     reciprocal + tensor_scalar mult
  5. PE transpose [128,16] -> [16,128] so the output DMA is contiguous.
"""

import numpy as np

B, S, H = 64, 2048, 1024
N_CORES = 8
P = 128

_PROG_CACHE = {}


def _build_program(bl, s, h):
    """Build the per-core Bass program for bl local batches, seq s, hidden h."""
    from contextlib import ExitStack

    import concourse.bass as bass
    import concourse.mybir as mybir
    import concourse.tile as tile
    from concourse import bacc, bass_isa
    from concourse.masks import make_identity

    f32 = mybir.dt.float32
    st = s // P          # s-tiles per batch
    kc = (2 * h) // P    # contraction chunks for energy matmul
    hn = (h + 511) // 512  # n-chunks of <=512 for matmul free dim
    assert h % 512 == 0 and s % P == 0 and (2 * h) % P == 0

    bf16 = mybir.dt.bfloat16
    nc = bacc.Bacc("TRN2", target_bir_lowering=False, debug=False)

    enc_d = nc.dram_tensor("enc", [bl, s, h], f32, kind="ExternalInput")
    # host-pretiled hi/lo bf16 split of csd.T: [p, kci, 2, b]
    csdt_d = nc.dram_tensor("csdt", [P, kc, 2, bl], bf16, kind="ExternalInput")
    # host-pretiled hi/lo bf16 split of W.T: [p, kci, 2, hh]
    wt_d = nc.dram_tensor("wt", [P, kc, 2, h], bf16, kind="ExternalInput")
    bias_d = nc.dram_tensor("bias", [1, 2, h], bf16, kind="ExternalInput")
    out_d = nc.dram_tensor("out", [bl, s], f32, kind="ExternalOutput")

    enc_ap = enc_d.ap()
    out_ap = out_d.ap()

    with tile.TileContext(nc) as tc, ExitStack() as ctx:
        singles = ctx.enter_context(tc.tile_pool(name="singles", bufs=1))
        wt_pool = ctx.enter_context(tc.tile_pool(name="wt", bufs=16))
        enc_pool = ctx.enter_context(tc.tile_pool(name="encp", bufs=16))
        eb_pool = ctx.enter_context(tc.tile_pool(name="eb", bufs=1))
        col_pool = ctx.enter_context(tc.tile_pool(name="cols", bufs=3))
        sm_pool = ctx.enter_context(tc.tile_pool(name="sm", bufs=4))
        dummy_pool = ctx.enter_context(tc.tile_pool(name="dummy", bufs=2))
        ps_en = ctx.enter_context(tc.tile_pool(name="ps_en", bufs=1, space="PSUM"))
        ps_bc = ctx.enter_context(tc.tile_pool(name="ps_bc", bufs=2, space="PSUM"))
        ps_t = ctx.enter_context(tc.tile_pool(name="ps_t", bufs=2, space="PSUM"))

        # ---- constants / small loads ----
        with tc.high_priority():
            csdt_sb = singles.tile([P, kc, 2, bl], bf16)
            nc.sync.dma_start(csdt_sb, csdt_d.ap())
            bias_sb = singles.tile([1, 2, h], bf16)
            nc.sync.dma_start(bias_sb, bias_d.ap())
        identity = singles.tile([P, P], f32)
        make_identity(nc, identity)
        ones1bl = singles.tile([1, bl], bf16)
        nc.vector.memset(ones1bl, 1.0)
        # one-hot selectors: sel[k, b, m] = 1.0 iff k == b  (lhsT for bcast)
        sel = singles.tile([bl, bl, P], bf16)
        nc.gpsimd.memset(sel, 0.0)
        # iota = k*1 + b*(-1) + m*0; where != 0 keep 0.0, where == 0 fill 1.0
        nc.gpsimd.affine_select(
            out=sel,
            in_=sel,
            compare_op=mybir.AluOpType.not_equal,
            fill=1.0,
            base=0,
            pattern=[[-1, bl], [0, P]],
            channel_multiplier=1,
        )

        # ---- energy = csd @ W.T + bias  -> PSUM [bl, h] ----
        # bf16 hi/lo 3-pass decomposition: exact to ~2^-17 rel, runs the PE
        # at 1 cycle/row instead of fp32's ~4.
        en_ps = ps_en.tile([bl, h], f32)
        for k in range(kc):
            wt_t = wt_pool.tile([P, 2, h], bf16)
            with tc.high_priority():
                nc.sync.dma_start(wt_t, wt_d.ap()[:, k, :, :])
            for n in range(hn):
                nsl = slice(n * 512, (n + 1) * 512)
                for ci, wi in ((0, 0), (0, 1), (1, 0)):
                    nc.tensor.matmul(
                        en_ps[:, nsl],
                        csdt_sb[:, k, ci, :],
                        wt_t[:, wi, nsl],
                        start=(k == 0 and ci == 0 and wi == 0),
                        stop=False,
                    )
        for n in range(hn):
            nsl = slice(n * 512, (n + 1) * 512)
            for wi in (0, 1):
                nc.tensor.matmul(
                    en_ps[:, nsl],
                    ones1bl,
                    bias_sb[:, wi, nsl],
                    start=False,
                    stop=(wi == 1),
                )
        # hi/lo split of energy (read straight from PSUM) for the bf16
        # broadcast matmuls
        e_hi = singles.tile([bl, h], bf16)
        nc.vector.tensor_copy(e_hi, en_ps)
        e_lo = singles.tile([bl, h], bf16)
        nc.vector.tensor_sub(e_lo, en_ps, e_hi)

        # ---- broadcast energy rows to all 128 partitions ----
        eb_tiles = []
        for b in range(bl):
            bc_ps = ps_bc.tile([P, h], f32)
            for n in range(hn):
                nsl = slice(n * 512, (n + 1) * 512)
                nc.tensor.matmul(
                    bc_ps[:, nsl], sel[:, b, :], e_hi[:, nsl],
                    start=True, stop=False,
                )
                nc.tensor.matmul(
                    bc_ps[:, nsl], sel[:, b, :], e_lo[:, nsl],
                    start=False, stop=True,
                )
            eb = eb_pool.tile([P, h], f32, tag=f"eb{b}")
            nc.scalar.copy(eb, bc_ps)
            eb_tiles.append(eb)

        # ---- main loop: fused multiply+reduce per [128, h] tile ----
        for b in range(bl):
            cols = col_pool.tile([P, st], f32)
            for t in range(st):
                et = enc_pool.tile([P, h], f32)
                nc.sync.dma_start(et, enc_ap[b, t * P : (t + 1) * P, :])
                dummy = dummy_pool.tile([P, 1], f32)
                nc.vector.scalar_tensor_tensor(
                    out=dummy.broadcast_to((P, h)),
                    in0=et,
                    scalar=1.0,
                    in1=eb_tiles[b],
                    op0=mybir.AluOpType.mult,
                    op1=mybir.AluOpType.mult,
                    accum_out=cols[:, t : t + 1],
                )

            # ---- softmax over the batch's s-dim ([128, st] tile) ----
            mx = sm_pool.tile([P, 1], f32)
            nc.vector.tensor_reduce(
                mx, cols, mybir.AxisListType.X, mybir.AluOpType.max
            )
            nc.gpsimd.partition_all_reduce(mx, mx, P, bass_isa.ReduceOp.max)
            negmx = sm_pool.tile([P, 1], f32)
            nc.vector.tensor_scalar_mul(negmx, mx, -1.0)
            ex = sm_pool.tile([P, st], f32)
            sume = sm_pool.tile([P, 1], f32)
            nc.scalar.activation(
                ex,
                cols,
                mybir.ActivationFunctionType.Exp,
                bias=negmx,
                scale=1.0,
                accum_out=sume,
            )
            nc.gpsimd.partition_all_reduce(sume, sume, P, bass_isa.ReduceOp.add)
            rec = sm_pool.tile([P, 1], f32)
            nc.vector.reciprocal(rec, sume)
            prob = sm_pool.tile([P, st], f32)
            nc.vector.tensor_scalar_mul(prob, ex, rec)

            # ---- transpose [128, st] -> [st, 128] and store contiguously ----
            pt = ps_t.tile([st, P], f32)
            nc.tensor.transpose(pt, prob, identity)
            outt = sm_pool.tile([st, P], f32)
            nc.scalar.copy(outt, pt)
            nc.scalar.dma_start(
                out_ap[b].rearrange("(t c) -> t c", c=P), outt
            )

    nc.compile()
    return nc


def _build_program_cc(bl, s, h, n_cores):
    """v2: W sharded over h across cores; energy slices exchanged by AllGather.

    Per-core inputs:
      enc       [bl, s, h]        this core's batches
      csdt_all  [P, kc, B]        csd.T tiled, ALL batches (replicated)
      wt_shard  [P, kc, hs]       W.T tiled, this core's h-slice (hs = h/n_cores)
      bias_shard [1, hs]          bias slice for this core's h-slice
      sel       [B, bl, P]        one-hot: sel[k, lb, m] = (k == core*bl + lb)
    """
    from contextlib import ExitStack

    import concourse.bass as bass
    import concourse.mybir as mybir
    import concourse.tile as tile
    from concourse import bacc, bass_isa
    from concourse.masks import make_identity

    f32 = mybir.dt.float32
    bt = bl * n_cores       # total batches
    hs = h // n_cores       # h-slice per core
    st = s // P
    kc = (2 * h) // P
    hn = (h + 511) // 512
    assert h % 512 == 0 and s % P == 0 and h % n_cores == 0

    nc = bacc.Bacc(
        "TRN2", target_bir_lowering=False, debug=False, num_devices=n_cores
    )

    enc_d = nc.dram_tensor("enc", [bl, s, h], f32, kind="ExternalInput")
    csdt_d = nc.dram_tensor("csdt_all", [P, kc, bt], f32, kind="ExternalInput")
    wt_d = nc.dram_tensor("wt_shard", [P, kc, hs], f32, kind="ExternalInput")
    bias_d = nc.dram_tensor("bias_shard", [1, hs], f32, kind="ExternalInput")
    sel_d = nc.dram_tensor("sel", [bt, bl, P], f32, kind="ExternalInput")
    out_d = nc.dram_tensor("out", [bl, s], f32, kind="ExternalOutput")

    enc_ap = enc_d.ap()
    out_ap = out_d.ap()

    with tile.TileContext(nc) as tc, ExitStack() as ctx:
        singles = ctx.enter_context(tc.tile_pool(name="singles", bufs=1))
        enc_pool = ctx.enter_context(tc.tile_pool(name="encp", bufs=8))
        eb_pool = ctx.enter_context(tc.tile_pool(name="eb", bufs=1))
        col_pool = ctx.enter_context(tc.tile_pool(name="cols", bufs=3))
        sm_pool = ctx.enter_context(tc.tile_pool(name="sm", bufs=4))
        dummy_pool = ctx.enter_context(tc.tile_pool(name="dummy", bufs=2))
        dram = ctx.enter_context(tc.tile_pool(name="dram", bufs=1, space="DRAM"))
        ps_en = ctx.enter_context(tc.tile_pool(name="ps_en", bufs=1, space="PSUM"))
        ps_bc = ctx.enter_context(tc.tile_pool(name="ps_bc", bufs=2, space="PSUM"))
        ps_t = ctx.enter_context(tc.tile_pool(name="ps_t", bufs=2, space="PSUM"))

        # ---- small loads (critical path first) ----
        wt_sb = singles.tile([P, kc, hs], f32)
        nc.sync.dma_start(wt_sb, wt_d.ap())
        csdt_sb = singles.tile([P, kc, bt], f32)
        nc.sync.dma_start(csdt_sb, csdt_d.ap())
        bias_sb = singles.tile([1, hs], f32)
        nc.sync.dma_start(bias_sb, bias_d.ap())
        sel_sb = singles.tile([bt, bl, P], f32)
        nc.sync.dma_start(sel_sb, sel_d.ap())
        identity = singles.tile([P, P], f32)
        make_identity(nc, identity)
        ones1 = singles.tile([1, bt], f32)
        nc.vector.memset(ones1, 1.0)

        # ---- energy slice for ALL batches: [bt, hs] ----
        en_ps = ps_en.tile([bt, hs], f32)
        for k in range(kc):
            nc.tensor.matmul(
                en_ps,
                csdt_sb[:, k, :],
                wt_sb[:, k, :],
                start=(k == 0),
                stop=False,
            )
        nc.tensor.matmul(en_ps, ones1, bias_sb, start=False, stop=True)
        en_slice = singles.tile([bt, hs], f32)
        nc.scalar.copy(en_slice, en_ps)

        # ---- AllGather energy slices -> full energy for all batches ----
        cc_in = dram.tile([bt, hs], f32)
        cc_out = dram.tile([n_cores, bt, hs], f32)
        nc.sync.dma_start(cc_in[:], en_slice)
        nc.gpsimd.collective_compute(
            "AllGather",
            mybir.AluOpType.bypass,
            replica_groups=[list(range(n_cores))],
            ins=[cc_in.opt()],
            outs=[cc_out.opt()],
        )
        # energy_all[b, hc, hh] == energy[b, hc*hs + hh]
        energy_all = singles.tile([bt, n_cores, hs], f32)
        nc.sync.dma_start(
            energy_all, cc_out[:].rearrange("hc b hh -> b hc hh")
        )
        energy_flat = energy_all.rearrange("b hc hh -> b (hc hh)")

        # ---- broadcast this core's batch energies to all 128 partitions ----
        eb_tiles = []
        for b in range(bl):
            bc_ps = ps_bc.tile([P, h], f32)
            for n in range(hn):
                nsl = slice(n * 512, (n + 1) * 512)
                nc.tensor.matmul(
                    bc_ps[:, nsl], sel_sb[:, b, :], energy_flat[:, nsl],
                    start=True, stop=True,
                )
            eb = eb_pool.tile([P, h], f32, tag=f"eb{b}")
            nc.scalar.copy(eb, bc_ps)
            eb_tiles.append(eb)

        # ---- main loop: fused multiply+reduce per [128, h] tile ----
        for b in range(bl):
            cols = col_pool.tile([P, st], f32)
            for t in range(st):
                et = enc_pool.tile([P, h], f32)
                nc.sync.dma_start(et, enc_ap[b, t * P : (t + 1) * P, :])
                dummy = dummy_pool.tile([P, 1], f32)
                nc.vector.scalar_tensor_tensor(
                    out=dummy.broadcast_to((P, h)),
                    in0=et,
                    scalar=1.0,
                    in1=eb_tiles[b],
                    op0=mybir.AluOpType.mult,
                    op1=mybir.AluOpType.mult,
                    accum_out=cols[:, t : t + 1],
                )

            # ---- softmax ----
            mx = sm_pool.tile([P, 1], f32)
            nc.vector.tensor_reduce(
                mx, cols, mybir.AxisListType.X, mybir.AluOpType.max
            )
            nc.gpsimd.partition_all_reduce(mx, mx, P, bass_isa.ReduceOp.max)
            negmx = sm_pool.tile([P, 1], f32)
            nc.vector.tensor_scalar_mul(negmx, mx, -1.0)
            ex = sm_pool.tile([P, st], f32)
            sume = sm_pool.tile([P, 1], f32)
            nc.scalar.activation(
                ex,
                cols,
                mybir.ActivationFunctionType.Exp,
                bias=negmx,
                scale=1.0,
                accum_out=sume,
            )
            nc.gpsimd.partition_all_reduce(sume, sume, P, bass_isa.ReduceOp.add)
            rec = sm_pool.tile([P, 1], f32)
            nc.vector.reciprocal(rec, sume)
            prob = sm_pool.tile([P, st], f32)
            nc.vector.tensor_scalar_mul(prob, ex, rec)

            pt = ps_t.tile([st, P], f32)
            nc.tensor.transpose(pt, prob, identity)
            outt = sm_pool.tile([st, P], f32)
            nc.scalar.copy(outt, pt)
            nc.scalar.dma_start(out_ap[b].rearrange("(t c) -> t c", c=P), outt)

    nc.compile()
    return nc


def _prep_inputs_cc(char_state_decoder, encoder_outputs, W, b):
    bl = B // N_CORES
    hs = H // N_CORES
    kc = (2 * H) // P
    csd = np.ascontiguousarray(np.asarray(char_state_decoder, dtype=np.float32))
    enc = np.ascontiguousarray(np.asarray(encoder_outputs, dtype=np.float32))
    Wf = np.asarray(W, dtype=np.float32)
    bias = np.asarray(b, dtype=np.float32)
    wt = np.ascontiguousarray(Wf.T.reshape(kc, P, H).transpose(1, 0, 2))  # [P,kc,H]
    csdt_all = np.ascontiguousarray(csd.T.reshape(kc, P, B).transpose(1, 0, 2))
    in_maps = []
    for i in range(N_CORES):
        sel = np.zeros((B, bl, P), dtype=np.float32)
        for lb in range(bl):
            sel[i * bl + lb, lb, :] = 1.0
        in_maps.append(
            {
                "enc": enc[i * bl : (i + 1) * bl],
                "csdt_all": csdt_all,
                "wt_shard": np.ascontiguousarray(wt[:, :, i * hs : (i + 1) * hs]),
                "bias_shard": np.ascontiguousarray(
                    bias[i * hs : (i + 1) * hs].reshape(1, hs)
                ),
                "sel": sel,
            }
        )
    return in_maps


def _get_program(bl, s, h):
    key = (bl, s, h)
    if key not in _PROG_CACHE:
        _PROG_CACHE[key] = _build_program(bl, s, h)
    return _PROG_CACHE[key]


def _get_program_cc():
    key = "cc"
    if key not in _PROG_CACHE:
        _PROG_CACHE[key] = _build_program_cc(B // N_CORES, S, H, N_CORES)
    return _PROG_CACHE[key]


def _hilo(a):
    """Split f32 array into (hi, lo) bf16 pair with hi+lo ~= a (~2^-17 rel)."""
    import ml_dtypes

    hi = a.astype(ml_dtypes.bfloat16)
    lo = (a - hi.astype(np.float32)).astype(ml_dtypes.bfloat16)
    return hi, lo


def _prep_inputs(char_state_decoder, encoder_outputs, W, b):
    """Host-side layout prep (slicing/transpose/dtype-split only, no math)."""
    bl = B // N_CORES
    kc = (2 * H) // P
    csd = np.ascontiguousarray(np.asarray(char_state_decoder, dtype=np.float32))
    enc = np.ascontiguousarray(np.asarray(encoder_outputs, dtype=np.float32))
    Wf = np.asarray(W, dtype=np.float32)
    bias = np.asarray(b, dtype=np.float32).reshape(1, H)
    bias_hl = np.ascontiguousarray(np.stack(_hilo(bias), axis=1))  # [1, 2, H]
    # wt[p, kci, hh] = W[hh, kci*128 + p]
    wt = Wf.T.reshape(kc, P, H).transpose(1, 0, 2)  # [P, kc, H] f32
    wt_hl = np.ascontiguousarray(np.stack(_hilo(wt), axis=2))  # [P, kc, 2, H]
    in_maps = []
    for i in range(N_CORES):
        csd_l = csd[i * bl : (i + 1) * bl]  # [bl, 2H]
        csdt = csd_l.T.reshape(kc, P, bl).transpose(1, 0, 2)  # [P, kc, bl]
        csdt_hl = np.ascontiguousarray(np.stack(_hilo(csdt), axis=2))
        in_maps.append(
            {
                "enc": enc[i * bl : (i + 1) * bl],
                "csdt": csdt_hl,
                "wt": wt_hl,
                "bias": bias_hl,
            }
        )
    return in_maps


def run_on_cores(char_state_decoder, encoder_outputs, W, b, trace=False, cc=False):
    """Run the sharded kernel on 8 cores; returns (out [B,1,S], BassKernelResults)."""
    from concourse.bass_utils import run_bass_kernel_spmd

    if cc:
        nc = _get_program_cc()
        in_maps = _prep_inputs_cc(char_state_decoder, encoder_outputs, W, b)
    else:
        nc = _get_program(B // N_CORES, S, H)
        in_maps = _prep_inputs(char_state_decoder, encoder_outputs, W, b)
    res = run_bass_kernel_spmd(
        nc, in_maps, core_ids=list(range(N_CORES)), trace=trace
    )
    out = np.concatenate([r["out"] for r in res.results], axis=0)  # [B, S]
    return out.reshape(B, 1, S).astype(np.float32), res


def kernel(char_state_decoder, encoder_outputs, W, b):
    out, _ = run_on_cores(char_state_decoder, encoder_outputs, W, b, trace=False)
    return out


# revision 19
# speedup vs baseline: 1.1593x; 1.1593x over previous
"""Trainium2 Bass kernel for batched attention-energy softmax.

Computes, for B=64, S=2048, H=1024:
    energy = char_state_decoder @ W.T + b            # [B, H]
    attn   = softmax(einsum("bsh,bh->bs", encoder_outputs, energy), axis=1)
    return attn[:, None, :]                          # [B, 1, S]

Sharding: data-parallel over batch across 8 NeuronCores (8 batches/core).
W/b are replicated; host prep is layout-only (transpose/tile/bf16 hi-lo
split, no math).

Per-core device program:
  1. PE: energy[8,1024] = csd_local @ W.T + b via bf16 hi/lo 3-pass matmuls
     (exact to ~2^-17 rel; runs PE at 1 cyc/row instead of fp32's ~4),
     K-accumulated in PSUM.
  2. PE: broadcast energy row b to 128 partitions with one-hot selector
     matmuls (selectors exact in bf16; energy re-split hi/lo on DVE).
  3. DVE: for each [128s, 1024h] encoder tile, ONE fused
     scalar_tensor_tensor (elementwise mult + free-dim sum) -> attn
     energies column [128, 1]. This streams the 64 MB/core encoder tensor
     exactly once; the kernel is HBM-bandwidth bound (~360 GB/s/core).
  4. Softmax per batch over [128, 16] (s = col*128 + partition):
     DVE free-dim max -> GPSIMD partition all-reduce -> ScalarE Exp with
     bias=-max and accum_out partial sums -> GPSIMD all-reduce -> DVE
     reciprocal + tensor_scalar mult
  5. PE transpose [128,16] -> [16,128] so the output DMA is contiguous.

(Note: tensor_tensor_reduce crashes this runtime's NeuronCores;
scalar_tensor_tensor with op0=mult/scalar=1 is the working equivalent.)
"""

import numpy as np

B, S, H = 64, 2048, 1024
N_CORES = 8
P = 128

_PROG_CACHE = {}


def _build_program(bl, s, h):
    """Build the per-core Bass program for bl local batches, seq s, hidden h."""
    from contextlib import ExitStack

    import concourse.bass as bass
    import concourse.mybir as mybir
    import concourse.tile as tile
    from concourse import bacc, bass_isa
    from concourse.masks import make_identity

    f32 = mybir.dt.float32
    st = s // P          # s-tiles per batch
    kc = (2 * h) // P    # contraction chunks for energy matmul
    hn = (h + 511) // 512  # n-chunks of <=512 for matmul free dim
    assert h % 512 == 0 and s % P == 0 and (2 * h) % P == 0

    bf16 = mybir.dt.bfloat16
    nc = bacc.Bacc("TRN2", target_bir_lowering=False, debug=False)

    enc_d = nc.dram_tensor("enc", [bl, s, h], f32, kind="ExternalInput")
    # host-pretiled hi/lo bf16 split of csd.T: [p, kci, 2, b]
    csdt_d = nc.dram_tensor("csdt", [P, kc, 2, bl], bf16, kind="ExternalInput")
    # host-pretiled hi/lo bf16 split of W.T: [p, kci, 2, hh]
    wt_d = nc.dram_tensor("wt", [P, kc, 2, h], bf16, kind="ExternalInput")
    bias_d = nc.dram_tensor("bias", [1, 2, h], bf16, kind="ExternalInput")
    out_d = nc.dram_tensor("out", [bl, s], f32, kind="ExternalOutput")

    enc_ap = enc_d.ap()
    out_ap = out_d.ap()

    with tile.TileContext(nc) as tc, ExitStack() as ctx:
        singles = ctx.enter_context(tc.tile_pool(name="singles", bufs=1))
        wt_pool = ctx.enter_context(tc.tile_pool(name="wt", bufs=16))
        enc_pool = ctx.enter_context(tc.tile_pool(name="encp", bufs=16))
        eb_pool = ctx.enter_context(tc.tile_pool(name="eb", bufs=1))
        col_pool = ctx.enter_context(tc.tile_pool(name="cols", bufs=3))
        sm_pool = ctx.enter_context(tc.tile_pool(name="sm", bufs=4))
        dummy_pool = ctx.enter_context(tc.tile_pool(name="dummy", bufs=2))
        ps_en = ctx.enter_context(tc.tile_pool(name="ps_en", bufs=1, space="PSUM"))
        ps_bc = ctx.enter_context(tc.tile_pool(name="ps_bc", bufs=2, space="PSUM"))
        ps_t = ctx.enter_context(tc.tile_pool(name="ps_t", bufs=2, space="PSUM"))

        # ---- constants / small loads ----
        csdt_sb = singles.tile([P, kc, 2, bl], bf16)
        nc.sync.dma_start(csdt_sb, csdt_d.ap())
        bias_sb = singles.tile([1, 2, h], bf16)
        nc.sync.dma_start(bias_sb, bias_d.ap())
        identity = singles.tile([P, P], f32)
        make_identity(nc, identity)
        ones1bl = singles.tile([1, bl], bf16)
        nc.vector.memset(ones1bl, 1.0)
        # one-hot selectors: sel[k, b, m] = 1.0 iff k == b  (lhsT for bcast)
        sel = singles.tile([bl, bl, P], bf16)
        nc.gpsimd.memset(sel, 0.0)
        # iota = k*1 + b*(-1) + m*0; where != 0 keep 0.0, where == 0 fill 1.0
        nc.gpsimd.affine_select(
            out=sel,
            in_=sel,
            compare_op=mybir.AluOpType.not_equal,
            fill=1.0,
            base=0,
            pattern=[[-1, bl], [0, P]],
            channel_multiplier=1,
        )

        # ---- energy = csd @ W.T + bias  -> PSUM [bl, h] ----
        # bf16 hi/lo 3-pass decomposition: exact to ~2^-17 rel, runs the PE
        # at 1 cycle/row instead of fp32's ~4.
        en_ps = ps_en.tile([bl, h], f32)
        for k in range(kc):
            wt_t = wt_pool.tile([P, 2, h], bf16)
            nc.sync.dma_start(wt_t, wt_d.ap()[:, k, :, :])
            for n in range(hn):
                nsl = slice(n * 512, (n + 1) * 512)
                for ci, wi in ((0, 0), (0, 1), (1, 0)):
                    nc.tensor.matmul(
                        en_ps[:, nsl],
                        csdt_sb[:, k, ci, :],
                        wt_t[:, wi, nsl],
                        start=(k == 0 and ci == 0 and wi == 0),
                        stop=False,
                    )
        for n in range(hn):
            nsl = slice(n * 512, (n + 1) * 512)
            for wi in (0, 1):
                nc.tensor.matmul(
                    en_ps[:, nsl],
                    ones1bl,
                    bias_sb[:, wi, nsl],
                    start=False,
                    stop=(wi == 1),
                )
        energy_sb = singles.tile([bl, h], f32)
        nc.scalar.copy(energy_sb, en_ps)
        # hi/lo split of energy for the bf16 broadcast matmuls
        e_hi = singles.tile([bl, h], bf16)
        nc.vector.tensor_copy(e_hi, energy_sb)
        e_lo = singles.tile([bl, h], bf16)
        nc.vector.tensor_sub(e_lo, energy_sb, e_hi)

        # ---- broadcast energy rows to all 128 partitions ----
        eb_tiles = []
        for b in range(bl):
            bc_ps = ps_bc.tile([P, h], f32)
            for n in range(hn):
                nsl = slice(n * 512, (n + 1) * 512)
                nc.tensor.matmul(
                    bc_ps[:, nsl], sel[:, b, :], e_hi[:, nsl],
                    start=True, stop=False,
                )
                nc.tensor.matmul(
                    bc_ps[:, nsl], sel[:, b, :], e_lo[:, nsl],
                    start=False, stop=True,
                )
            eb = eb_pool.tile([P, h], f32, tag=f"eb{b}")
            nc.scalar.copy(eb, bc_ps)
            eb_tiles.append(eb)

        # ---- main loop: fused multiply+reduce per [128, h] tile ----
        for b in range(bl):
            cols = col_pool.tile([P, st], f32)
            for t in range(st):
                et = enc_pool.tile([P, h], f32)
                nc.sync.dma_start(et, enc_ap[b, t * P : (t + 1) * P, :])
                dummy = dummy_pool.tile([P, 1], f32)
                nc.vector.scalar_tensor_tensor(
                    out=dummy.broadcast_to((P, h)),
                    in0=et,
                    scalar=1.0,
                    in1=eb_tiles[b],
                    op0=mybir.AluOpType.mult,
                    op1=mybir.AluOpType.mult,
                    accum_out=cols[:, t : t + 1],
                )

            # ---- softmax over the batch's s-dim ([128, st] tile) ----
            mx = sm_pool.tile([P, 1], f32)
            nc.vector.tensor_reduce(
                mx, cols, mybir.AxisListType.X, mybir.AluOpType.max
            )
            nc.gpsimd.partition_all_reduce(mx, mx, P, bass_isa.ReduceOp.max)
            negmx = sm_pool.tile([P, 1], f32)
            nc.vector.tensor_scalar_mul(negmx, mx, -1.0)
            ex = sm_pool.tile([P, st], f32)
            sume = sm_pool.tile([P, 1], f32)
            nc.scalar.activation(
                ex,
                cols,
                mybir.ActivationFunctionType.Exp,
                bias=negmx,
                scale=1.0,
                accum_out=sume,
            )
            nc.gpsimd.partition_all_reduce(sume, sume, P, bass_isa.ReduceOp.add)
            rec = sm_pool.tile([P, 1], f32)
            nc.vector.reciprocal(rec, sume)
            prob = sm_pool.tile([P, st], f32)
            nc.vector.tensor_scalar_mul(prob, ex, rec)

            # ---- transpose [128, st] -> [st, 128] and store contiguously ----
            pt = ps_t.tile([st, P], f32)
            nc.tensor.transpose(pt, prob, identity)
            outt = sm_pool.tile([st, P], f32)
            nc.scalar.copy(outt, pt)
            nc.scalar.dma_start(
                out_ap[b].rearrange("(t c) -> t c", c=P), outt
            )

    nc.compile()
    return nc


def _build_program_cc(bl, s, h, n_cores):
    """v2: W sharded over h across cores; energy slices exchanged by AllGather.

    Per-core inputs:
      enc       [bl, s, h]        this core's batches
      csdt_all  [P, kc, B]        csd.T tiled, ALL batches (replicated)
      wt_shard  [P, kc, hs]       W.T tiled, this core's h-slice (hs = h/n_cores)
      bias_shard [1, hs]          bias slice for this core's h-slice
      sel       [B, bl, P]        one-hot: sel[k, lb, m] = (k == core*bl + lb)
    """
    from contextlib import ExitStack

    import concourse.bass as bass
    import concourse.mybir as mybir
    import concourse.tile as tile
    from concourse import bacc, bass_isa
    from concourse.masks import make_identity

    f32 = mybir.dt.float32
    bt = bl * n_cores       # total batches
    hs = h // n_cores       # h-slice per core
    st = s // P
    kc = (2 * h) // P
    hn = (h + 511) // 512
    assert h % 512 == 0 and s % P == 0 and h % n_cores == 0

    nc = bacc.Bacc(
        "TRN2", target_bir_lowering=False, debug=False, num_devices=n_cores
    )

    enc_d = nc.dram_tensor("enc", [bl, s, h], f32, kind="ExternalInput")
    csdt_d = nc.dram_tensor("csdt_all", [P, kc, bt], f32, kind="ExternalInput")
    wt_d = nc.dram_tensor("wt_shard", [P, kc, hs], f32, kind="ExternalInput")
    bias_d = nc.dram_tensor("bias_shard", [1, hs], f32, kind="ExternalInput")
    sel_d = nc.dram_tensor("sel", [bt, bl, P], f32, kind="ExternalInput")
    out_d = nc.dram_tensor("out", [bl, s], f32, kind="ExternalOutput")

    enc_ap = enc_d.ap()
    out_ap = out_d.ap()

    with tile.TileContext(nc) as tc, ExitStack() as ctx:
        singles = ctx.enter_context(tc.tile_pool(name="singles", bufs=1))
        enc_pool = ctx.enter_context(tc.tile_pool(name="encp", bufs=8))
        eb_pool = ctx.enter_context(tc.tile_pool(name="eb", bufs=1))
        col_pool = ctx.enter_context(tc.tile_pool(name="cols", bufs=3))
        sm_pool = ctx.enter_context(tc.tile_pool(name="sm", bufs=4))
        dummy_pool = ctx.enter_context(tc.tile_pool(name="dummy", bufs=2))
        dram = ctx.enter_context(tc.tile_pool(name="dram", bufs=1, space="DRAM"))
        ps_en = ctx.enter_context(tc.tile_pool(name="ps_en", bufs=1, space="PSUM"))
        ps_bc = ctx.enter_context(tc.tile_pool(name="ps_bc", bufs=2, space="PSUM"))
        ps_t = ctx.enter_context(tc.tile_pool(name="ps_t", bufs=2, space="PSUM"))

        # ---- small loads (critical path first) ----
        wt_sb = singles.tile([P, kc, hs], f32)
        nc.sync.dma_start(wt_sb, wt_d.ap())
        csdt_sb = singles.tile([P, kc, bt], f32)
        nc.sync.dma_start(csdt_sb, csdt_d.ap())
        bias_sb = singles.tile([1, hs], f32)
        nc.sync.dma_start(bias_sb, bias_d.ap())
        sel_sb = singles.tile([bt, bl, P], f32)
        nc.sync.dma_start(sel_sb, sel_d.ap())
        identity = singles.tile([P, P], f32)
        make_identity(nc, identity)
        ones1 = singles.tile([1, bt], f32)
        nc.vector.memset(ones1, 1.0)

        # ---- energy slice for ALL batches: [bt, hs] ----
        en_ps = ps_en.tile([bt, hs], f32)
        for k in range(kc):
            nc.tensor.matmul(
                en_ps,
                csdt_sb[:, k, :],
                wt_sb[:, k, :],
                start=(k == 0),
                stop=False,
            )
        nc.tensor.matmul(en_ps, ones1, bias_sb, start=False, stop=True)
        en_slice = singles.tile([bt, hs], f32)
        nc.scalar.copy(en_slice, en_ps)

        # ---- AllGather energy slices -> full energy for all batches ----
        cc_in = dram.tile([bt, hs], f32)
        cc_out = dram.tile([n_cores, bt, hs], f32)
        nc.sync.dma_start(cc_in[:], en_slice)
        nc.gpsimd.collective_compute(
            "AllGather",
            mybir.AluOpType.bypass,
            replica_groups=[list(range(n_cores))],
            ins=[cc_in.opt()],
            outs=[cc_out.opt()],
        )
        # energy_all[b, hc, hh] == energy[b, hc*hs + hh]
        energy_all = singles.tile([bt, n_cores, hs], f32)
        nc.sync.dma_start(
            energy_all, cc_out[:].rearrange("hc b hh -> b hc hh")
        )
        energy_flat = energy_all.rearrange("b hc hh -> b (hc hh)")

        # ---- broadcast this core's batch energies to all 128 partitions ----
        eb_tiles = []
        for b in range(bl):
            bc_ps = ps_bc.tile([P, h], f32)
            for n in range(hn):
                nsl = slice(n * 512, (n + 1) * 512)
                nc.tensor.matmul(
                    bc_ps[:, nsl], sel_sb[:, b, :], energy_flat[:, nsl],
                    start=True, stop=True,
                )
            eb = eb_pool.tile([P, h], f32, tag=f"eb{b}")
            nc.scalar.copy(eb, bc_ps)
            eb_tiles.append(eb)

        # ---- main loop: fused multiply+reduce per [128, h] tile ----
        for b in range(bl):
            cols = col_pool.tile([P, st], f32)
            for t in range(st):
                et = enc_pool.tile([P, h], f32)
                nc.sync.dma_start(et, enc_ap[b, t * P : (t + 1) * P, :])
                dummy = dummy_pool.tile([P, 1], f32)
                nc.vector.scalar_tensor_tensor(
                    out=dummy.broadcast_to((P, h)),
                    in0=et,
                    scalar=1.0,
                    in1=eb_tiles[b],
                    op0=mybir.AluOpType.mult,
                    op1=mybir.AluOpType.mult,
                    accum_out=cols[:, t : t + 1],
                )

            # ---- softmax ----
            mx = sm_pool.tile([P, 1], f32)
            nc.vector.tensor_reduce(
                mx, cols, mybir.AxisListType.X, mybir.AluOpType.max
            )
            nc.gpsimd.partition_all_reduce(mx, mx, P, bass_isa.ReduceOp.max)
            negmx = sm_pool.tile([P, 1], f32)
            nc.vector.tensor_scalar_mul(negmx, mx, -1.0)
            ex = sm_pool.tile([P, st], f32)
            sume = sm_pool.tile([P, 1], f32)
            nc.scalar.activation(
                ex,
                cols,
                mybir.ActivationFunctionType.Exp,
                bias=negmx,
                scale=1.0,
                accum_out=sume,
            )
            nc.gpsimd.partition_all_reduce(sume, sume, P, bass_isa.ReduceOp.add)
            rec = sm_pool.tile([P, 1], f32)
            nc.vector.reciprocal(rec, sume)
            prob = sm_pool.tile([P, st], f32)
            nc.vector.tensor_scalar_mul(prob, ex, rec)

            pt = ps_t.tile([st, P], f32)
            nc.tensor.transpose(pt, prob, identity)
            outt = sm_pool.tile([st, P], f32)
            nc.scalar.copy(outt, pt)
            nc.scalar.dma_start(out_ap[b].rearrange("(t c) -> t c", c=P), outt)

    nc.compile()
    return nc


def _prep_inputs_cc(char_state_decoder, encoder_outputs, W, b):
    bl = B // N_CORES
    hs = H // N_CORES
    kc = (2 * H) // P
    csd = np.ascontiguousarray(np.asarray(char_state_decoder, dtype=np.float32))
    enc = np.ascontiguousarray(np.asarray(encoder_outputs, dtype=np.float32))
    Wf = np.asarray(W, dtype=np.float32)
    bias = np.asarray(b, dtype=np.float32)
    wt = np.ascontiguousarray(Wf.T.reshape(kc, P, H).transpose(1, 0, 2))  # [P,kc,H]
    csdt_all = np.ascontiguousarray(csd.T.reshape(kc, P, B).transpose(1, 0, 2))
    in_maps = []
    for i in range(N_CORES):
        sel = np.zeros((B, bl, P), dtype=np.float32)
        for lb in range(bl):
            sel[i * bl + lb, lb, :] = 1.0
        in_maps.append(
            {
                "enc": enc[i * bl : (i + 1) * bl],
                "csdt_all": csdt_all,
                "wt_shard": np.ascontiguousarray(wt[:, :, i * hs : (i + 1) * hs]),
                "bias_shard": np.ascontiguousarray(
                    bias[i * hs : (i + 1) * hs].reshape(1, hs)
                ),
                "sel": sel,
            }
        )
    return in_maps


def _get_program(bl, s, h):
    key = (bl, s, h)
    if key not in _PROG_CACHE:
        _PROG_CACHE[key] = _build_program(bl, s, h)
    return _PROG_CACHE[key]


def _get_program_cc():
    key = "cc"
    if key not in _PROG_CACHE:
        _PROG_CACHE[key] = _build_program_cc(B // N_CORES, S, H, N_CORES)
    return _PROG_CACHE[key]


def _hilo(a):
    """Split f32 array into (hi, lo) bf16 pair with hi+lo ~= a (~2^-17 rel)."""
    import ml_dtypes

    hi = a.astype(ml_dtypes.bfloat16)
    lo = (a - hi.astype(np.float32)).astype(ml_dtypes.bfloat16)
    return hi, lo


def _prep_inputs(char_state_decoder, encoder_outputs, W, b):
    """Host-side layout prep (slicing/transpose/dtype-split only, no math)."""
    bl = B // N_CORES
    kc = (2 * H) // P
    csd = np.ascontiguousarray(np.asarray(char_state_decoder, dtype=np.float32))
    enc = np.ascontiguousarray(np.asarray(encoder_outputs, dtype=np.float32))
    Wf = np.asarray(W, dtype=np.float32)
    bias = np.asarray(b, dtype=np.float32).reshape(1, H)
    bias_hl = np.ascontiguousarray(np.stack(_hilo(bias), axis=1))  # [1, 2, H]
    # wt[p, kci, hh] = W[hh, kci*128 + p]
    wt = Wf.T.reshape(kc, P, H).transpose(1, 0, 2)  # [P, kc, H] f32
    wt_hl = np.ascontiguousarray(np.stack(_hilo(wt), axis=2))  # [P, kc, 2, H]
    in_maps = []
    for i in range(N_CORES):
        csd_l = csd[i * bl : (i + 1) * bl]  # [bl, 2H]
        csdt = csd_l.T.reshape(kc, P, bl).transpose(1, 0, 2)  # [P, kc, bl]
        csdt_hl = np.ascontiguousarray(np.stack(_hilo(csdt), axis=2))
        in_maps.append(
            {
                "enc": enc[i * bl : (i + 1) * bl],
                "csdt": csdt_hl,
                "wt": wt_hl,
                "bias": bias_hl,
            }
        )
    return in_maps


def run_on_cores(char_state_decoder, encoder_outputs, W, b, trace=False, cc=False):
    """Run the sharded kernel on 8 cores; returns (out [B,1,S], BassKernelResults)."""
    from concourse.bass_utils import run_bass_kernel_spmd

    if cc:
        nc = _get_program_cc()
        in_maps = _prep_inputs_cc(char_state_decoder, encoder_outputs, W, b)
    else:
        nc = _get_program(B // N_CORES, S, H)
        in_maps = _prep_inputs(char_state_decoder, encoder_outputs, W, b)
    res = run_bass_kernel_spmd(
        nc, in_maps, core_ids=list(range(N_CORES)), trace=trace
    )
    out = np.concatenate([r["out"] for r in res.results], axis=0)  # [B, S]
    return out.reshape(B, 1, S).astype(np.float32), res


def kernel(char_state_decoder, encoder_outputs, W, b):
    out, _ = run_on_cores(char_state_decoder, encoder_outputs, W, b, trace=False)
    return out
